# revision 23
# baseline (speedup 1.0000x reference)
"""Trainium2 Bass kernel for nn_ExecPolicyNetwork.

Reference computation:
    mask = exec_mask[job_idx]                       # [200000] bool
    idx  = nonzero(mask, size=num_exec_acts)[0]     # first K active indices, ascending
    a    = idx/200000                               # [K] f32
    score_k = W3 @ tanh(W2 @ tanh(c1 + a_k*v1) + b2) + b3
  where c1 = W1[:, :259] @ concat(x_dag, h_dag, h_glob) + b1 and v1 = W1[:, 259].

Because every score depends on its executor index only through the scalar
a = idx/200000, the kernel evaluates the MLP densely on the full uniform
index grid: the executor axis (200000) is split into 8 slices of 25000, one
per NeuronCore.  On the uniform grid the h1 stage collapses to a single
fused activation per 512-point tile (tanh(iota*(v1/E) + bias_col)); the h2
stage packs two 64-row tanh tiles into one 128-partition activation; the W3
stage is a 2-column packed matmul.  Each core returns its 25600 dense
scores; the host applies the boolean mask while unsharding (this DMA stack
only supports row-granular indirection, so element compaction stays host
side) and concatenates the ragged results.
"""

import sys

if "/opt/trn_rl_repo" not in sys.path:
    sys.path.insert(0, "/opt/trn_rl_repo")

import numpy as np

E = 200000          # num executors
NCORES = 8
SHARD = E // NCORES  # 25000 dense points per core
P = 128
NPTS = 25600        # padded points per core (25 tiles x 1024)
T = NPTS // 1024    # 25 macro tiles of 1024 points
NH = 2 * T          # 50 half-tiles of 512 points

# column layout of the packed f32 constant tensor [128, NCONST].
# c1 = W1[:, :259] @ base + b1 and the per-half-tile bias columns
# biasmat[:, j] = c1 + v1*(shard_off + 512*j)/E are folded host-side
# (a [128, 259] matvec per job), so the device preamble is just one DMA
# plus DVE staging.
C_W2T = 0           # [:, 0:64]     W2.T
C_W3P = 64          # [:, 64:66]    W3 pack (rows 0:64 col 0, rows 64:128 col 1)
C_B2P = 66          # [:, 66]       [b2; b2]
C_B3P = 67          # [0:2, 67]     b3 (twice)
C_V1OE = 68         # [:, 68]       v1/E
C_BIAS = 69         # [:, 69:119]   biasmat
NCONST = 119

_NC_CACHE = {}


def _build_nc():
    import concourse.mybir as mybir
    import concourse.tile as tile
    from concourse import bacc

    dt = mybir.dt
    Act = mybir.ActivationFunctionType

    nc = bacc.Bacc()
    f32 = dt.float32

    CONSTS = nc.dram_tensor("CONSTS", [P, NCONST], f32, kind="ExternalInput")
    OUT = nc.dram_tensor("OUT", [T, 2, 512], f32, kind="ExternalOutput")

    with tile.TileContext(nc) as tc:
        with (
            tc.tile_pool(name="const", bufs=1) as cpool,
            tc.tile_pool(name="work", bufs=5) as wpool,
            tc.tile_pool(name="sc", bufs=3) as scpool,
            tc.tile_pool(name="pz", bufs=3, space="PSUM") as pzpool,
            tc.tile_pool(name="ps", bufs=3, space="PSUM") as pspool,
        ):
            # ---------------- loads & generated constants ----------------
            consts = cpool.tile([P, NCONST], f32)
            nc.sync.dma_start(consts[:], CONSTS[:])

            iota512p = cpool.tile([128, 512], f32)
            nc.gpsimd.iota(iota512p[:], [[1, 512]], base=0, channel_multiplier=0,
                           allow_small_or_imprecise_dtypes=True)

            # DVE staging: every matmul operand becomes DVE-written so each
            # matmul carries at most ONE semaphore wait (HW limit), and each
            # DVE/ACT instruction introduces at most one new upstream sem.
            iota512 = cpool.tile([128, 512], f32)
            nc.vector.tensor_copy(iota512[:], iota512p[:])
            # h1-critical constants first so the first ACT op starts early
            v1oE = cpool.tile([128, 1], f32)
            nc.vector.tensor_copy(v1oE[:], consts[:, C_V1OE:C_V1OE + 1])
            biasmat = cpool.tile([128, NH], f32)
            nc.vector.tensor_copy(biasmat[:], consts[:, C_BIAS:C_BIAS + NH])
            w2t = cpool.tile([128, 64], f32)
            nc.vector.tensor_copy(w2t[:], consts[:, C_W2T:C_W2T + 64])
            w3p = cpool.tile([128, 2], f32)
            nc.vector.tensor_copy(w3p[:], consts[:, C_W3P:C_W3P + 2])
            b2d = cpool.tile([128, 1], f32)
            nc.vector.tensor_copy(b2d[:], consts[:, C_B2P:C_B2P + 1])
            b3d = cpool.tile([2, 1], f32)
            nc.vector.tensor_copy(b3d[:], consts[0:2, C_B3P:C_B3P + 1])

            # one tiny PE op on DVE-staged operands so later matmuls carry a
            # single already-observed semaphore
            warm_ps = pzpool.tile([1, 64], f32, tag="z")
            nc.tensor.matmul(warm_ps[:], w2t[:, 0:1], w2t[:], start=True,
                             stop=True)

            # ---------------- dense MLP (software-pipelined) ----------------
            # The W3-stage matmul of tile t-1 is issued after tile t's W2
            # matmuls, so the PE never waits for the ACT h2 round-trip.
            # Tile T-1 only covers SHARD - (T-1)*1024 = 424 real points, all
            # in its first 512-half; the rest is padding and is skipped.
            LASTW = SHARD - (T - 1) * 1024

            def w3_stage(t, h2p, w):
                sp = pspool.tile([2, 512], f32, tag="s")
                nc.tensor.matmul(sp[0:1, :w] if w < 512 else sp[:],
                                 w3p[0:64, 0:1] if w < 512 else w3p[:],
                                 h2p[0:64, :w] if w < 512 else h2p[:],
                                 start=True, stop=True)
                sc2 = scpool.tile([2, 512], f32, tag="sc2")
                if w < 512:
                    nc.vector.tensor_scalar_add(sc2[0:1, :w], sp[0:1, :w],
                                                b3d[0:1, :])
                    nc.sync.dma_start(OUT[t, 0:1, :w], sc2[0:1, :w])
                else:
                    nc.vector.tensor_scalar_add(sc2[:], sp[:], b3d[:])
                    nc.sync.dma_start(OUT[t], sc2[:])

            pending = []              # (t, h2p, width) awaiting their W3 stage
            DEPTH = 2
            for t in range(T - 1):
                h1a = wpool.tile([128, 512], f32, tag="h1a")
                nc.scalar.activation(h1a[:], iota512[:], Act.Tanh,
                                     bias=biasmat[:, 2 * t:2 * t + 1], scale=v1oE[:])
                h1b = wpool.tile([128, 512], f32, tag="h1b")
                nc.scalar.activation(h1b[:], iota512[:], Act.Tanh,
                                     bias=biasmat[:, 2 * t + 1:2 * t + 2],
                                     scale=v1oE[:])
                zp = pzpool.tile([128, 512], f32, tag="z")
                nc.tensor.matmul(zp[0:64, :], w2t[:], h1a[:], start=True, stop=True)
                nc.tensor.matmul(zp[64:128, :], w2t[:], h1b[:], start=True, stop=True)
                if len(pending) >= DEPTH:
                    w3_stage(*pending.pop(0))
                h2p = wpool.tile([128, 512], f32, tag="h2")
                nc.scalar.activation(h2p[:], zp[:], Act.Tanh, bias=b2d[:])
                pending.append((t, h2p, 512))
            t = T - 1
            h1a = wpool.tile([128, 512], f32, tag="h1a")
            nc.scalar.activation(h1a[:, :LASTW], iota512[:, :LASTW], Act.Tanh,
                                 bias=biasmat[:, 2 * t:2 * t + 1], scale=v1oE[:])
            zp = pzpool.tile([128, 512], f32, tag="z")
            nc.tensor.matmul(zp[0:64, :LASTW], w2t[:], h1a[:, :LASTW],
                             start=True, stop=True)
            h2p = wpool.tile([128, 512], f32, tag="h2")
            nc.scalar.activation(h2p[0:64, :LASTW], zp[0:64, :LASTW], Act.Tanh,
                                 bias=b2d[0:64, :])
            pending.append((t, h2p, LASTW))
            for p in pending:
                w3_stage(*p)
    nc.compile()
    return nc


def get_nc():
    if "nc" not in _NC_CACHE:
        _NC_CACHE["nc"] = _build_nc()
    return _NC_CACHE["nc"]


def make_in_maps(x, ptr, exec_mask, h_dag_all, h_glob, W1, b1, W2, b2, W3, b3,
                 job_idx):
    """Host-side sharding / layout prep. Returns in_maps."""
    job = int(job_idx)

    base = np.zeros(260, np.float32)
    base[:3] = np.asarray(x)[int(np.asarray(ptr)[job]), :3]
    base[3:131] = np.asarray(h_dag_all)[job]
    base[131:259] = np.asarray(h_glob)[0]

    W1 = np.asarray(W1, np.float32)
    W2 = np.asarray(W2, np.float32)
    W3 = np.asarray(W3, np.float32)

    v1 = W1[:, 259]
    c1 = (W1[:, :259] @ base[:259] + np.asarray(b1, np.float32)).astype(np.float32)

    consts0 = np.zeros((P, NCONST), np.float32)
    consts0[:, C_W2T:C_W2T + 64] = W2.T
    consts0[0:64, C_W3P] = W3[0]
    consts0[64:128, C_W3P + 1] = W3[0]
    consts0[:, C_B2P] = np.concatenate([np.asarray(b2, np.float32)] * 2)
    consts0[0:2, C_B3P] = np.float32(np.asarray(b3).reshape(-1)[0])
    consts0[:, C_V1OE] = v1 / np.float32(E)

    in_maps = []
    for d in range(NCORES):
        consts = consts0.copy()
        brow = ((np.float32(d * SHARD) + 512.0 * np.arange(NH, dtype=np.float32))
                / np.float32(E))
        consts[:, C_BIAS:C_BIAS + NH] = c1[:, None] + v1[:, None] * brow[None, :]
        in_maps.append({"CONSTS": consts})
    return in_maps


def _score_of_zero(base, W1, b1, W2, b2, W3, b3):
    """Host fallback for reference padding rows (idx = 0) when the mask has
    fewer than num_exec_acts active entries.  Never triggered for the staged
    problem (exactly 120000 active)."""
    c1 = W1[:, :259].astype(np.float32) @ base[:259] + b1
    h = np.tanh(c1)
    h = np.tanh(W2 @ h + b2)
    return float(W3 @ h + b3)


def kernel(**inputs):
    from concourse.bass_utils import run_bass_kernel_spmd

    x = inputs["x"]
    ptr = inputs["ptr"]
    exec_mask = inputs["exec_mask"]
    h_dag_all = inputs["h_dag_all"]
    h_glob = inputs["h_glob"]
    W1 = np.asarray(inputs["W1"], np.float32)
    b1 = np.asarray(inputs["b1"], np.float32)
    W2 = np.asarray(inputs["W2"], np.float32)
    b2 = np.asarray(inputs["b2"], np.float32)
    W3 = np.asarray(inputs["W3"], np.float32)
    b3 = np.asarray(inputs["b3"], np.float32)
    job = int(inputs["job_idx"])
    K = int(inputs["num_exec_acts"])

    in_maps = make_in_maps(x, ptr, exec_mask, h_dag_all, h_glob,
                           W1, b1, W2, b2, W3, b3, job)
    nc = get_nc()
    res = run_bass_kernel_spmd(nc, in_maps, core_ids=list(range(NCORES)))

    dense = np.empty(E, np.float32)
    for d in range(NCORES):
        dense[d * SHARD:(d + 1) * SHARD] = res.results[d]["OUT"].ravel()[:SHARD]

    mask = np.asarray(exec_mask[job]).astype(bool).ravel()
    scores = dense[mask]

    if scores.shape[0] >= K:
        out = scores[:K]
    else:
        base = np.zeros(260, np.float32)
        base[:3] = np.asarray(x)[int(np.asarray(ptr)[job]), :3]
        base[3:131] = np.asarray(h_dag_all)[job]
        base[131:259] = np.asarray(h_glob)[0]
        pad_val = _score_of_zero(base, W1, b1, W2, b2, W3, b3)
        out = np.concatenate(
            [scores, np.full(K - scores.shape[0], pad_val, np.float32)])
    return np.ascontiguousarray(out.astype(np.float32))


# revision 25
# speedup vs baseline: 1.0118x; 1.0118x over previous
"""Trainium2 Bass kernel for nn_ExecPolicyNetwork.

Reference computation:
    mask = exec_mask[job_idx]                       # [200000] bool
    idx  = nonzero(mask, size=num_exec_acts)[0]     # first K active indices, ascending
    a    = idx/200000                               # [K] f32
    score_k = W3 @ tanh(W2 @ tanh(c1 + a_k*v1) + b2) + b3
  where c1 = W1[:, :259] @ concat(x_dag, h_dag, h_glob) + b1 and v1 = W1[:, 259].

Because every score depends on its executor index only through the scalar
a = idx/200000, the kernel evaluates the MLP densely on the full uniform
index grid: the executor axis (200000) is split into 8 slices of 25000, one
per NeuronCore.  On the uniform grid the h1 stage collapses to a single
fused activation per 512-point tile (tanh(iota*(v1/E) + bias_col)); the h2
stage packs two 64-row tanh tiles into one 128-partition activation; the W3
stage is a 2-column packed matmul.  Each core returns its 25600 dense
scores; the host applies the boolean mask while unsharding (this DMA stack
only supports row-granular indirection, so element compaction stays host
side) and concatenates the ragged results.
"""

import sys

if "/opt/trn_rl_repo" not in sys.path:
    sys.path.insert(0, "/opt/trn_rl_repo")

import numpy as np

E = 200000          # num executors
NCORES = 8
SHARD = E // NCORES  # 25000 dense points per core
P = 128
NPTS = 25600        # padded points per core (25 tiles x 1024)
T = NPTS // 1024    # 25 macro tiles of 1024 points
NH = 2 * T          # 50 half-tiles of 512 points

# column layout of the packed f32 constant tensor [128, NCONST].
# c1 = W1[:, :259] @ base + b1 and the per-half-tile bias columns
# biasmat[:, j] = c1 + v1*(shard_off + 512*j)/E are folded host-side
# (a [128, 259] matvec per job), so the device preamble is just one DMA
# plus DVE staging.
C_W2T = 0           # [:, 0:64]     W2.T
C_W3P = 64          # [:, 64:66]    W3 pack (rows 0:64 col 0, rows 64:128 col 1)
C_B2P = 66          # [:, 66]       [b2; b2]
C_B3P = 67          # [0:2, 67]     b3 (twice)
C_V1OE = 68         # [:, 68]       v1/E
C_BIAS = 69         # [:, 69:119]   biasmat
NCONST = 119

_NC_CACHE = {}


def _build_nc():
    import concourse.mybir as mybir
    import concourse.tile as tile
    from concourse import bacc

    dt = mybir.dt
    Act = mybir.ActivationFunctionType

    nc = bacc.Bacc()
    f32 = dt.float32

    CONSTS = nc.dram_tensor("CONSTS", [P, NCONST], f32, kind="ExternalInput")
    OUT = nc.dram_tensor("OUT", [T, 2, 512], f32, kind="ExternalOutput")

    with tile.TileContext(nc) as tc:
        with (
            tc.tile_pool(name="const", bufs=1) as cpool,
            tc.tile_pool(name="work", bufs=4) as wpool,
            tc.tile_pool(name="sc", bufs=3) as scpool,
            tc.tile_pool(name="pz", bufs=3, space="PSUM") as pzpool,
            tc.tile_pool(name="ps", bufs=3, space="PSUM") as pspool,
        ):
            # ---------------- loads & generated constants ----------------
            consts = cpool.tile([P, NCONST], f32)
            nc.sync.dma_start(consts[:], CONSTS[:])

            iota512p = cpool.tile([128, 512], f32)
            nc.gpsimd.iota(iota512p[:], [[1, 512]], base=0, channel_multiplier=0,
                           allow_small_or_imprecise_dtypes=True)

            # DVE staging: every matmul operand becomes DVE-written so each
            # matmul carries at most ONE semaphore wait (HW limit), and each
            # DVE/ACT instruction introduces at most one new upstream sem.
            iota512 = cpool.tile([128, 512], f32)
            nc.vector.tensor_copy(iota512[:], iota512p[:])
            # dummy tanh on a zero-dep tile: hoists the ~1.3us ACT table
            # load into the entry window, off the first real h1 critical path
            zz = cpool.tile([1, 1], f32)
            nc.vector.memset(zz[:], 0.0)
            acttab = cpool.tile([1, 1], f32)
            nc.scalar.activation(acttab[:], zz[:], Act.Tanh)
            # h1-critical constants first so the first ACT op starts early
            v1oE = cpool.tile([128, 1], f32)
            nc.vector.tensor_copy(v1oE[:], consts[:, C_V1OE:C_V1OE + 1])
            biasmat = cpool.tile([128, NH], f32)
            nc.vector.tensor_copy(biasmat[:], consts[:, C_BIAS:C_BIAS + NH])
            w2t = cpool.tile([128, 64], f32)
            nc.vector.tensor_copy(w2t[:], consts[:, C_W2T:C_W2T + 64])
            w3p = cpool.tile([128, 2], f32)
            nc.vector.tensor_copy(w3p[:], consts[:, C_W3P:C_W3P + 2])
            b2d = cpool.tile([128, 1], f32)
            nc.vector.tensor_copy(b2d[:], consts[:, C_B2P:C_B2P + 1])
            b3d = cpool.tile([2, 1], f32)
            nc.vector.tensor_copy(b3d[:], consts[0:2, C_B3P:C_B3P + 1])

            # one tiny PE op on DVE-staged operands so later matmuls carry a
            # single already-observed semaphore
            warm_ps = pzpool.tile([1, 64], f32, tag="z")
            nc.tensor.matmul(warm_ps[:], w2t[:, 0:1], w2t[:], start=True,
                             stop=True)

            # ---------------- dense MLP (software-pipelined) ----------------
            # The W3-stage matmul of tile t-1 is issued after tile t's W2
            # matmuls, so the PE never waits for the ACT h2 round-trip.
            # Tile T-1 only covers SHARD - (T-1)*1024 = 424 real points, all
            # in its first 512-half; the rest is padding and is skipped.
            LASTW = SHARD - (T - 1) * 1024

            def w3_stage(t, h2p, w):
                sp = pspool.tile([2, 512], f32, tag="s")
                nc.tensor.matmul(sp[0:1, :w] if w < 512 else sp[:],
                                 w3p[0:64, 0:1] if w < 512 else w3p[:],
                                 h2p[0:64, :w] if w < 512 else h2p[:],
                                 start=True, stop=True)
                sc2 = scpool.tile([2, 512], f32, tag="sc2")
                if w < 512:
                    nc.vector.tensor_scalar_add(sc2[0:1, :w], sp[0:1, :w],
                                                b3d[0:1, :])
                    nc.sync.dma_start(OUT[t, 0:1, :w], sc2[0:1, :w])
                else:
                    nc.vector.tensor_scalar_add(sc2[:], sp[:], b3d[:])
                    nc.sync.dma_start(OUT[t], sc2[:])

            pending = None            # (t, h2p, width) awaiting its W3 stage
            for t in range(T - 1):
                h1a = wpool.tile([128, 512], f32, tag="h1a")
                nc.scalar.activation(h1a[:], iota512[:], Act.Tanh,
                                     bias=biasmat[:, 2 * t:2 * t + 1], scale=v1oE[:])
                h1b = wpool.tile([128, 512], f32, tag="h1b")
                nc.scalar.activation(h1b[:], iota512[:], Act.Tanh,
                                     bias=biasmat[:, 2 * t + 1:2 * t + 2],
                                     scale=v1oE[:])
                zp = pzpool.tile([128, 512], f32, tag="z")
                nc.tensor.matmul(zp[0:64, :], w2t[:], h1a[:], start=True, stop=True)
                nc.tensor.matmul(zp[64:128, :], w2t[:], h1b[:], start=True, stop=True)
                if pending is not None:
                    w3_stage(*pending)
                h2p = wpool.tile([128, 512], f32, tag="h2")
                nc.scalar.activation(h2p[:], zp[:], Act.Tanh, bias=b2d[:])
                pending = (t, h2p, 512)
            t = T - 1
            h1a = wpool.tile([128, 512], f32, tag="h1a")
            nc.scalar.activation(h1a[:, :LASTW], iota512[:, :LASTW], Act.Tanh,
                                 bias=biasmat[:, 2 * t:2 * t + 1], scale=v1oE[:])
            zp = pzpool.tile([128, 512], f32, tag="z")
            nc.tensor.matmul(zp[0:64, :LASTW], w2t[:], h1a[:, :LASTW],
                             start=True, stop=True)
            if pending is not None:
                w3_stage(*pending)
            h2p = wpool.tile([128, 512], f32, tag="h2")
            nc.scalar.activation(h2p[0:64, :LASTW], zp[0:64, :LASTW], Act.Tanh,
                                 bias=b2d[0:64, :])
            w3_stage(t, h2p, LASTW)
    nc.compile()
    return nc


def get_nc():
    if "nc" not in _NC_CACHE:
        _NC_CACHE["nc"] = _build_nc()
    return _NC_CACHE["nc"]


def make_in_maps(x, ptr, exec_mask, h_dag_all, h_glob, W1, b1, W2, b2, W3, b3,
                 job_idx):
    """Host-side sharding / layout prep. Returns in_maps."""
    job = int(job_idx)

    base = np.zeros(260, np.float32)
    base[:3] = np.asarray(x)[int(np.asarray(ptr)[job]), :3]
    base[3:131] = np.asarray(h_dag_all)[job]
    base[131:259] = np.asarray(h_glob)[0]

    W1 = np.asarray(W1, np.float32)
    W2 = np.asarray(W2, np.float32)
    W3 = np.asarray(W3, np.float32)

    v1 = W1[:, 259]
    c1 = (W1[:, :259] @ base[:259] + np.asarray(b1, np.float32)).astype(np.float32)

    consts0 = np.zeros((P, NCONST), np.float32)
    consts0[:, C_W2T:C_W2T + 64] = W2.T
    consts0[0:64, C_W3P] = W3[0]
    consts0[64:128, C_W3P + 1] = W3[0]
    consts0[:, C_B2P] = np.concatenate([np.asarray(b2, np.float32)] * 2)
    consts0[0:2, C_B3P] = np.float32(np.asarray(b3).reshape(-1)[0])
    consts0[:, C_V1OE] = v1 / np.float32(E)

    in_maps = []
    for d in range(NCORES):
        consts = consts0.copy()
        brow = ((np.float32(d * SHARD) + 512.0 * np.arange(NH, dtype=np.float32))
                / np.float32(E))
        consts[:, C_BIAS:C_BIAS + NH] = c1[:, None] + v1[:, None] * brow[None, :]
        in_maps.append({"CONSTS": consts})
    return in_maps


def _score_of_zero(base, W1, b1, W2, b2, W3, b3):
    """Host fallback for reference padding rows (idx = 0) when the mask has
    fewer than num_exec_acts active entries.  Never triggered for the staged
    problem (exactly 120000 active)."""
    c1 = W1[:, :259].astype(np.float32) @ base[:259] + b1
    h = np.tanh(c1)
    h = np.tanh(W2 @ h + b2)
    return float(W3 @ h + b3)


def kernel(**inputs):
    from concourse.bass_utils import run_bass_kernel_spmd

    x = inputs["x"]
    ptr = inputs["ptr"]
    exec_mask = inputs["exec_mask"]
    h_dag_all = inputs["h_dag_all"]
    h_glob = inputs["h_glob"]
    W1 = np.asarray(inputs["W1"], np.float32)
    b1 = np.asarray(inputs["b1"], np.float32)
    W2 = np.asarray(inputs["W2"], np.float32)
    b2 = np.asarray(inputs["b2"], np.float32)
    W3 = np.asarray(inputs["W3"], np.float32)
    b3 = np.asarray(inputs["b3"], np.float32)
    job = int(inputs["job_idx"])
    K = int(inputs["num_exec_acts"])

    in_maps = make_in_maps(x, ptr, exec_mask, h_dag_all, h_glob,
                           W1, b1, W2, b2, W3, b3, job)
    nc = get_nc()
    res = run_bass_kernel_spmd(nc, in_maps, core_ids=list(range(NCORES)))

    dense = np.empty(E, np.float32)
    for d in range(NCORES):
        dense[d * SHARD:(d + 1) * SHARD] = res.results[d]["OUT"].ravel()[:SHARD]

    mask = np.asarray(exec_mask[job]).astype(bool).ravel()
    scores = dense[mask]

    if scores.shape[0] >= K:
        out = scores[:K]
    else:
        base = np.zeros(260, np.float32)
        base[:3] = np.asarray(x)[int(np.asarray(ptr)[job]), :3]
        base[3:131] = np.asarray(h_dag_all)[job]
        base[131:259] = np.asarray(h_glob)[0]
        pad_val = _score_of_zero(base, W1, b1, W2, b2, W3, b3)
        out = np.concatenate(
            [scores, np.full(K - scores.shape[0], pad_val, np.float32)])
    return np.ascontiguousarray(out.astype(np.float32))
